# revision 4
# baseline (speedup 1.0000x reference)
"""HashEmbeddingLayer Trainium2 kernel.

Strategy (8 NeuronCores):
  - Host folds the input-independent hash functions into the table:
    W2[id] = 0.25 * concat_h(W[(id*hash_a[h] + hash_b[h]) % BUCKET]),
    shape [VOCAB, 4*HIDDEN] f32. One 8KB contiguous row per vocab id
    keeps every gather descriptor big enough to run at the HBM roofline
    (measured ~347 GB/s vs ~167 GB/s with 2KB row descriptors).
  - Tokens are sorted by id and split into 8 equal chunks of 4096; core
    c receives chunk c and only the W2 row range that chunk touches
    (~16K rows, ~131 MB) - this cuts host->device staging 8x vs
    replicating the table, and makes the device gather near-sequential.
    The shard base row is forced even so local-id parity == id parity.
  - Device (per core): 32 tiles of 128 tokens; per tile one indirect
    DMA gathers 128 x 8KB rows (one row per partition, offset = local
    token id), the per-token hash signs
    s_h = 2*((id*sign_a[h]+sign_b[h]) % 2) - 1 are applied on ACT/DVE
    (sign_a even => constant sign folded into the add/subtract tree;
    sign_a odd => sign = parity of id, computed on DVE), the 4 chunks
    are summed on DVE, and the result is written back via HWDGE.
  - Host scatters each core's rows back to original token positions.
"""
import sys

for _p in ("/opt/trn_rl_repo", "/root/.axon_site/_ro/trn_rl_repo"):
    if _p not in sys.path:
        sys.path.append(_p)

import numpy as np
import concourse.bass as bass
import concourse.mybir as mybir
from concourse import tile
from concourse.vector_clock import ScopedClock
from concourse.bass_utils import run_bass_kernel_spmd

B, T = 8, 4096
VOCAB = 128000
BUCKET = 262144
HIDDEN = 512
NUM_HASH = 4
N_CORES = 8
P = 128
N_TILES = T // P  # 32

_MAX_WAITS = 1


def _split_multi_waits(nc):
    """This container's walrus rejects >1 sync wait per instruction.
    Move excess waits onto same-engine NoOp carriers inserted just before
    the over-subscribed instruction (engine program order is block order
    filtered by engine, so the carrier blocks the engine at the same
    point the original wait did)."""
    for func in nc.m.functions:
        for blk in func.blocks:
            insts = blk.instructions
            i = 0
            while i < len(insts):
                inst = insts[i]
                si = inst.sync_info
                waits = list(si.on_wait) if si is not None and si.on_wait else []
                if len(waits) > _MAX_WAITS:
                    si.on_wait = waits[-_MAX_WAITS:]
                    rest = waits[:-_MAX_WAITS]
                    carriers = []
                    for j in range(0, len(rest), _MAX_WAITS):
                        nop = mybir.InstNoOp(
                            name=nc.get_next_instruction_name(), ins=[], outs=[]
                        )
                        nop.engine = inst.engine
                        nop.sync_info = mybir.SyncInfo(
                            on_wait=rest[j:j + _MAX_WAITS], on_update=[]
                        )
                        carriers.append(nop)
                    insts[i:i] = carriers
                    i += len(carriers)
                i += 1


class _TileContext(tile.TileContext):
    def _drain_and_barrier(self, tick_clock, wait_clock):
        probe = self.nc.sync.nop(nofuse=True)
        wait_clock.add_sem_waits(
            probe.ins, ScopedClock({None: tick_clock.global_clock})
        )
        si = probe.ins.sync_info
        waits = list(si.on_wait) if si is not None and si.on_wait else []
        if len(waits) > _MAX_WAITS:
            si.on_wait = waits[:_MAX_WAITS]
            rest = waits[_MAX_WAITS:]
            for j in range(0, len(rest), _MAX_WAITS):
                extra = self.nc.sync.nop(nofuse=True)
                esi = extra.ins.sync_info
                if esi is None:
                    extra.ins.sync_info = mybir.SyncInfo(
                        on_wait=rest[j:j + _MAX_WAITS], on_update=[]
                    )
                else:
                    esi.on_wait = rest[j:j + _MAX_WAITS]
        self.nc.sync.drain()
        self.nc.all_engine_barrier()
        assert self.sems is not None
        popped = self.nc._tile_sem_poison_stack.pop()
        assert popped is self._sem_poison
        self.nc.clear_and_free_semaphores(list(self.sems.allocated().values()))
        self.nc.all_engine_barrier()

    def __exit__(self, *args):
        ret = super().__exit__(*args)
        _split_multi_waits(self.nc)
        return ret


def _build_program(sign_a, sign_b, n_rows, g_bufs=4, sc_bufs=6, acc_bufs=3):
    """Trace the per-core Bass program. sign_a/sign_b are python ints;
    n_rows is the per-core W2 shard height."""
    nc = bass.Bass("TRN2", target_bir_lowering=False, debug=False,
                   num_devices=N_CORES)
    ids_in = nc.dram_tensor("ids", [P, N_TILES], mybir.dt.int32,
                            kind="ExternalInput")
    w2_in = nc.dram_tensor("w2", [n_rows, NUM_HASH * HIDDEN],
                           mybir.dt.float32, kind="ExternalInput")
    out_d = nc.dram_tensor("out", [T, HIDDEN], mybir.dt.float32,
                           kind="ExternalOutput")

    # sign kind per hash: sign of (id*sa + sb) mod 2.
    # sa even -> constant sign; sa odd -> sign follows id parity.
    kinds = []  # 'dyn' | +1 | -1
    for h in range(NUM_HASH):
        if sign_a[h] % 2 == 1:
            kinds.append('dyn')
        else:
            kinds.append(1 if sign_b[h] % 2 == 1 else -1)

    with _TileContext(nc) as tc:
        with tc.tile_pool(name="consts", bufs=1) as cpool, \
             tc.tile_pool(name="g", bufs=g_bufs) as gpool, \
             tc.tile_pool(name="sc", bufs=sc_bufs) as scpool, \
             tc.tile_pool(name="acc", bufs=acc_bufs) as apool:
            ids_t = cpool.tile([P, N_TILES], mybir.dt.int32)
            nc.sync.dma_start(out=ids_t[:], in_=ids_in[:])

            # per-token dynamic signs: s_h = 2*((id + (sb&1)) & 1) - 1
            s_tiles = {}
            if any(k == 'dyn' for k in kinds):
                idlow = cpool.tile([P, N_TILES], mybir.dt.int32)
                nc.vector.tensor_scalar(
                    out=idlow[:], in0=ids_t[:], scalar1=1, scalar2=None,
                    op0=mybir.AluOpType.bitwise_and)
                for h in range(NUM_HASH):
                    if kinds[h] != 'dyn':
                        continue
                    bit = cpool.tile([P, N_TILES], mybir.dt.int32,
                                     tag=f"bit{h}")
                    nc.vector.tensor_scalar(
                        out=bit[:], in0=idlow[:],
                        scalar1=int(sign_b[h]) & 1, scalar2=None,
                        op0=mybir.AluOpType.add)
                    nc.vector.tensor_scalar(
                        out=bit[:], in0=bit[:], scalar1=1, scalar2=None,
                        op0=mybir.AluOpType.bitwise_and)
                    sf = cpool.tile([P, N_TILES], mybir.dt.float32,
                                    tag=f"sf{h}")
                    nc.vector.tensor_copy(out=sf[:], in_=bit[:])
                    s_h = cpool.tile([P, N_TILES], mybir.dt.float32,
                                     tag=f"s{h}")
                    nc.vector.tensor_scalar(
                        out=s_h[:], in0=sf[:], scalar1=2.0, scalar2=1.0,
                        op0=mybir.AluOpType.mult,
                        op1=mybir.AluOpType.subtract)
                    s_tiles[h] = s_h

            scale_eng = 0  # alternate dynamic-sign scales between ACT and DVE
            for t in range(N_TILES):
                g = gpool.tile([P, NUM_HASH * HIDDEN], mybir.dt.float32)
                nc.gpsimd.indirect_dma_start(
                    out=g[:], out_offset=None, in_=w2_in[:],
                    in_offset=bass.IndirectOffsetOnAxis(
                        ap=ids_t[:, t:t + 1], axis=0))

                pos, neg = [], []
                for h in range(NUM_HASH):
                    chunk = g[:, h * HIDDEN:(h + 1) * HIDDEN]
                    if kinds[h] == 'dyn':
                        d = scpool.tile([P, HIDDEN], mybir.dt.float32,
                                        tag=f"d{h}")
                        if scale_eng % 2 == 0:
                            nc.scalar.activation(
                                out=d[:], in_=chunk,
                                func=mybir.ActivationFunctionType.Copy,
                                scale=s_tiles[h][:, t:t + 1])
                        else:
                            nc.vector.tensor_scalar(
                                out=d[:], in0=chunk,
                                scalar1=s_tiles[h][:, t:t + 1], scalar2=None,
                                op0=mybir.AluOpType.mult)
                        scale_eng += 1
                        pos.append(d[:])
                    elif kinds[h] == 1:
                        pos.append(chunk)
                    else:
                        neg.append(chunk)

                acc = apool.tile([P, HIDDEN], mybir.dt.float32)
                if pos:
                    terms = [(ap, mybir.AluOpType.add) for ap in pos[1:]]
                    terms += [(ap, mybir.AluOpType.subtract) for ap in neg]
                    nc.vector.tensor_tensor(
                        out=acc[:], in0=pos[0], in1=terms[0][0],
                        op=terms[0][1])
                    for ap, op in terms[1:]:
                        nc.vector.tensor_tensor(
                            out=acc[:], in0=acc[:], in1=ap, op=op)
                else:
                    # all four signs constant -1: acc = -(n0+n1+n2+n3)
                    nc.vector.tensor_tensor(
                        out=acc[:], in0=neg[0], in1=neg[1],
                        op=mybir.AluOpType.add)
                    for ap in neg[2:]:
                        nc.vector.tensor_tensor(
                            out=acc[:], in0=acc[:], in1=ap,
                            op=mybir.AluOpType.add)
                    nc.vector.tensor_scalar(
                        out=acc[:], in0=acc[:], scalar1=-1.0, scalar2=None,
                        op0=mybir.AluOpType.mult)

                nc.sync.dma_start(out=out_d[t * P:(t + 1) * P, :], in_=acc[:])

    return nc


def _prepare_shards(input_ids, weight, hash_a, hash_b):
    """Sort tokens by id, split into 8 chunks, slice W2 per chunk."""
    flat_ids = input_ids.reshape(-1).astype(np.int64)
    order = np.argsort(flat_ids, kind="stable")
    ids_sorted = flat_ids[order].reshape(N_CORES, T)

    lo = ids_sorted[:, 0].copy()
    lo -= lo & 1  # even base keeps id parity in local ids
    span = ids_sorted[:, -1] - lo + 1
    n_rows = int(span.max())
    n_rows = min(-(-n_rows // 2048) * 2048, VOCAB)  # round up, stabilize NEFF

    # W2[id] = 0.25 * concat_h W[(id*a_h + b_h) % BUCKET], built per shard
    w2_shards = []
    ids_local = []
    for c in range(N_CORES):
        base = int(lo[c])
        hi = min(base + n_rows, VOCAB)
        vocab_ids = np.arange(base, hi, dtype=np.int64)
        buckets = (vocab_ids[:, None] * hash_a[None, :]
                   + hash_b[None, :]) % BUCKET
        shard = np.zeros((n_rows, NUM_HASH * HIDDEN), dtype=np.float32)
        shard[:hi - base] = weight[buckets.reshape(-1)].reshape(
            hi - base, NUM_HASH * HIDDEN)
        shard[:hi - base] *= 0.25
        w2_shards.append(shard)
        loc = (ids_sorted[c] - base).astype(np.int32)
        ids_local.append(np.ascontiguousarray(loc.reshape(N_TILES, P).T))
    return order, ids_local, w2_shards, n_rows


def kernel(input_ids, weight, hash_a, hash_b, sign_a, sign_b):
    input_ids = np.asarray(input_ids)
    weight = np.asarray(weight, dtype=np.float32)
    hash_a = np.asarray(hash_a).astype(np.int64)
    hash_b = np.asarray(hash_b).astype(np.int64)
    sign_a = np.asarray(sign_a).astype(np.int64)
    sign_b = np.asarray(sign_b).astype(np.int64)

    order, ids_local, w2_shards, n_rows = _prepare_shards(
        input_ids, weight, hash_a, hash_b)

    nc = _build_program([int(x) for x in sign_a], [int(x) for x in sign_b],
                        n_rows)

    in_maps = [{"ids": ids_local[c], "w2": w2_shards[c]}
               for c in range(N_CORES)]
    res = run_bass_kernel_spmd(nc, in_maps, core_ids=list(range(N_CORES)))

    out_flat = np.empty((B * T, HIDDEN), dtype=np.float32)
    for c in range(N_CORES):
        out_flat[order[c * T:(c + 1) * T]] = res.results[c]["out"]
    return out_flat.reshape(B, T, HIDDEN)
